# revision 1
# baseline (speedup 1.0000x reference)
"""Trainium2 Bass kernel for nn_EncodingNetwork (gnn_message_passing).

Math (exact collapse of the reference):
    enc       = x @ W_enc.T + b_enc                    [N=200, D=1024]
    cm[w]     = class-mean of enc                      [20, D]
    gm        = mean(enc, axis=0) = mean(cm, axis=0)   [D]
    per_class = cm @ Wl.T + gm @ Wr.T + b_rel          [20, 2D]
    out       = gaussian * per_class[:, D:] + per_class[:, :D]

Host-side constant folds (weight-only, data-independent):
  - 1/N_SUPPORT folded into W_enc so the class-mean selector stays 1.0.
  - b_enc folded into the relation bias: per_class is affine in b_enc, so
    b_rel_eff = b_rel + (Wl + Wr) @ b_enc and the device never adds b_enc
    (removes 8 vector adds from the post-DMA critical path of v1).

Device pipeline per core (output columns split 128/core; everything in
transposed [feature, class] layout so contractions sit on partitions):
  xm^T  (8 k-chunks x 2 mm) : class-sums of x via selector matmul
  cm^T  (8 passes x 8 mm)   : W_enc^T-chunk @ xm^T, chasing wcm DMA chunks
  rel   (8 k-tiles x 4 mm)  : W_rel chunks @ cm^T into 4 psum accumulators
  epilogue (6 vector ops)   : fold rhs row-means, bias, gaussian combine
  out   [128, 20] f32 -> host gathers the 8 column slices.

DMA strategy (hard-won):
  - Only 8 HWDGE DMA semaphores exist (bass_rust.NUM_HWDGE_SEMS); a 9th
    in-flight DMA's issue *waits* on reusing an old semaphore, collapsing
    ring pipelining.  So: exactly 8 load DMAs (out-store reuses the
    long-retired first sem).  The small epilogue constants ride the xsb
    blob as bf16 and are cast to f32 on device.
  - Both HW-DGE rings (sync=SP, scalar=Act) carry ~1.8 MB each in
    consumption order; wcm split in 4 quarters so cm passes chase the
    stream instead of gating on a whole 2 MB blob (whose last semaphore
    increment can straggle ~1.6 us behind the data).
"""

import numpy as np

import concourse.bass as bass  # noqa: F401
import concourse.tile as tile
from concourse import bacc, mybir
from concourse.bass import ts
from concourse.bass_utils import run_bass_kernel_spmd

N_WAY = 20
N_SUPPORT = 10
N = N_WAY * N_SUPPORT  # 200
D = 1024
NC = 8
SL = D // NC  # 128 output columns per core
KT = D // 128  # 8 contraction tiles
XW = D + N_WAY  # x | selector columns, per 128-row tile
NSM = 22  # small-constant columns appended to the xsb blob
XB = 2 * XW + NSM
F32 = mybir.dt.float32
BF16 = mybir.dt.bfloat16

USE_ALLGATHER = False


def _build_nc(use_ag: bool) -> bacc.Bacc:
    nc = bacc.Bacc("TRN2", target_bir_lowering=False, debug=False, num_devices=NC)

    def mm(out, lhsT, rhs, **kw):
        nc.tensor.matmul(out, lhsT, rhs, **kw)

    xs_h = nc.declare_dram_parameter("xsb", [128, XB], BF16, isOutput=False)
    wcm_h = nc.declare_dram_parameter("wcm", [128, KT * D], BF16, isOutput=False)
    wrel_h = nc.declare_dram_parameter("wrel", [128, KT * 512], BF16, isOutput=False)
    out_h = nc.declare_dram_parameter("out", [128, N_WAY], F32, isOutput=True)

    with tile.TileContext(nc) as tc:
        with (
            tc.tile_pool(name="sbuf", bufs=1) as sb,
            tc.tile_pool(name="psx", bufs=2, space="PSUM") as psx,
            tc.tile_pool(name="psc", bufs=2, space="PSUM") as psc,
            tc.tile_pool(name="psr", bufs=1, space="PSUM") as psr,
        ):
            # ---- 8 load DMAs, ~512 KB each, alternating rings in
            # consumption order.  Empirically the best batch size: bigger
            # batches serialize the early stream (descriptor batches are
            # fetched single-file at ~1.3 us per 128 descriptors), while
            # fewer/larger batches also coarsen consumer gating; 8 batches
            # stays exactly within the NUM_HWDGE_SEMS=8 pool so no issue
            # ever waits on semaphore reuse (the out-store reuses the
            # long-retired first sem).
            xs = sb.tile([128, XB], BF16, tag="xs")
            wcm_all = sb.tile([128, KT * D], BF16, tag="wcm")
            wrel_all = sb.tile([128, KT * 512], BF16, tag="wrel")
            nc.sync.dma_start(xs[:, :XW], xs_h[:, :XW])
            nc.scalar.dma_start(xs[:, XW:], xs_h[:, XW:])
            nc.sync.dma_start(wcm_all[:, 0:2048], wcm_h[:, 0:2048])
            nc.scalar.dma_start(wcm_all[:, 2048:4096], wcm_h[:, 2048:4096])
            nc.sync.dma_start(wcm_all[:, 4096:6144], wcm_h[:, 4096:6144])
            nc.scalar.dma_start(wcm_all[:, 6144:8192], wcm_h[:, 6144:8192])
            nc.sync.dma_start(wrel_all[:, 0:2048], wrel_h[:, 0:2048])
            nc.scalar.dma_start(wrel_all[:, 2048:4096], wrel_h[:, 2048:4096])

            # small constants -> f32 once
            smw = sb.tile([128, NSM], F32, tag="smw")
            nc.vector.tensor_copy(smw[:], xs[:, 2 * XW :])

            # ---- stage 1: xm^T chunks [128, 20] = x^T @ selector
            xm_sb = sb.tile([128, KT * N_WAY], BF16, tag="xm")
            for t in range(KT):
                p = psx.tile([128, N_WAY], F32, tag="xm_ps", name=f"pxm{t}")
                for i in range(2):
                    mm(
                        p[:],
                        xs[:, i * XW + t * 128 : i * XW + (t + 1) * 128],
                        xs[:, i * XW + D : i * XW + D + N_WAY],
                        start=(i == 0),
                        stop=(i == 1),
                    )
                nc.vector.tensor_copy(xm_sb[:, ts(t, N_WAY)], p[:])

            # ---- stage 2: cm^T passes chase the wcm quarters (pass order
            # follows the chunk landing order above)
            cmf = sb.tile([128, KT * N_WAY], BF16, tag="cmf")
            for t in range(KT):
                pcm = psc.tile([128, N_WAY], F32, tag="cm_ps", name=f"pcm{t}")
                for kt in range(KT):
                    mm(
                        pcm[:],
                        wcm_all[:, t * D + kt * 128 : t * D + (kt + 1) * 128],
                        xm_sb[:, ts(kt, N_WAY)],
                        start=(kt == 0),
                        stop=(kt == KT - 1),
                    )
                nc.vector.tensor_copy(cmf[:, ts(t, N_WAY)], pcm[:])

            # ---- stage 3: the four rel products against cm^T
            pm = psr.tile([128, N_WAY], F32, tag="pm")
            pmR = psr.tile([128, N_WAY], F32, tag="pmR")
            pstd = psr.tile([128, N_WAY], F32, tag="pstd")
            pstdR = psr.tile([128, N_WAY], F32, tag="pstdR")
            for kt in range(KT):
                rhs = cmf[:, ts(kt, N_WAY)]
                st, sp = (kt == 0), (kt == KT - 1)
                o = kt * 512
                mm(pm[:], wrel_all[:, o : o + 128], rhs, start=st, stop=sp)
                mm(pmR[:], wrel_all[:, o + 128 : o + 256], rhs, start=st, stop=sp)
                mm(pstd[:], wrel_all[:, o + 256 : o + 384], rhs, start=st, stop=sp)
                mm(pstdR[:], wrel_all[:, o + 384 : o + 512], rhs, start=st, stop=sp)

            # ---- stage 4: fold rhs row-means + biases, gaussian combine
            rm = sb.tile([128, 1], F32, tag="rm")
            rs = sb.tile([128, 1], F32, tag="rs")
            nc.vector.reduce_sum(rm[:], pmR[:], axis=mybir.AxisListType.X)
            nc.vector.reduce_sum(rs[:], pstdR[:], axis=mybir.AxisListType.X)
            bias_m = sb.tile([128, 1], F32, tag="bias_m")
            bias_s = sb.tile([128, 1], F32, tag="bias_s")
            nc.vector.tensor_scalar(
                bias_m[:], rm[:], 1.0 / N_WAY, smw[:, 0:1],
                op0=mybir.AluOpType.mult, op1=mybir.AluOpType.add,
            )
            nc.vector.tensor_scalar(
                bias_s[:], rs[:], 1.0 / N_WAY, smw[:, 1:2],
                op0=mybir.AluOpType.mult, op1=mybir.AluOpType.add,
            )
            t_sg = sb.tile([128, N_WAY], F32, tag="t_sg")
            nc.vector.scalar_tensor_tensor(
                t_sg[:], pstd[:], bias_s[:], smw[:, 2:22],
                op0=mybir.AluOpType.add, op1=mybir.AluOpType.mult,
            )
            out_sb = sb.tile([128, N_WAY], F32, tag="out")
            nc.vector.scalar_tensor_tensor(
                out_sb[:], pm[:], bias_m[:], t_sg[:],
                op0=mybir.AluOpType.add, op1=mybir.AluOpType.add,
            )
            nc.sync.dma_start(out_h[:], out_sb[:])

    nc.finalize()
    return nc


_NC_CACHE: dict = {}


def _get_nc(use_ag: bool) -> bacc.Bacc:
    key = use_ag
    if key not in _NC_CACHE:
        _NC_CACHE[key] = _build_nc(use_ag)
    return _NC_CACHE[key]


def _make_in_maps(x, W_enc, b_enc, W_rel, b_rel, gaussian, use_ag):
    import ml_dtypes

    nd = ml_dtypes.bfloat16
    # class-mean scaling folded into W_enc (fp32, before cast)
    W_enc = W_enc / np.float32(N_SUPPORT)
    # b_enc folded into the relation bias
    b_rel_eff = (
        b_rel + W_rel[:, :D].astype(np.float64) @ b_enc
        + W_rel[:, D:].astype(np.float64) @ b_enc
    ).astype(np.float32)

    # xsb: [128, XB] -- two 128-row tiles of [x | selector], then smalls
    xsf = np.zeros((128, XB), np.float32)
    xs2 = np.zeros((2, 128, XW), np.float32)
    xs2[:, :, :D].reshape(256, D)[:N] = x
    sel = np.zeros((N, N_WAY), np.float32)
    sel[np.arange(N), np.arange(N) // N_SUPPORT] = 1.0
    xs2[:, :, D : D + N_WAY].reshape(256, N_WAY)[:N] = sel
    xsf[:, : 2 * XW] = xs2.transpose(1, 0, 2).reshape(128, -1)

    # t-major blocks: wcm[p, t*D + kt*128 + j] = W_enc[t*128+j, kt*128+p]
    wcm = (
        np.ascontiguousarray(W_enc.T)
        .reshape(KT, 128, KT, 128)
        .transpose(1, 2, 0, 3)
        .reshape(128, KT * D)
    )
    wcm = np.ascontiguousarray(wcm).astype(nd)

    in_maps = []
    for c in range(NC):
        s = slice(c * SL, (c + 1) * SL)
        s2 = slice(D + c * SL, D + (c + 1) * SL)
        blk = np.empty((KT, 128, 512), np.float32)
        for i, m in enumerate(
            (W_rel[s, :D], W_rel[s, D:], W_rel[s2, :D], W_rel[s2, D:])
        ):
            blk[:, :, i * 128 : (i + 1) * 128] = (
                np.ascontiguousarray(m.T).reshape(KT, 128, SL)
            )
        wrel = blk.transpose(1, 0, 2).reshape(128, KT * 512)

        xc = xsf.copy()
        xc[:, 2 * XW + 0] = b_rel_eff[s]
        xc[:, 2 * XW + 1] = b_rel_eff[s2]
        xc[:, 2 * XW + 2 :] = gaussian[:, s].T
        in_maps.append(
            {
                "xsb": xc.astype(nd),
                "wcm": wcm,
                "wrel": np.ascontiguousarray(wrel).astype(nd),
            }
        )
    return in_maps


def run(inputs: dict, trace: bool = False, use_ag: bool = USE_ALLGATHER):
    x = np.asarray(inputs["x_support"], np.float32)
    W_enc = np.asarray(inputs["W_enc"], np.float32)
    b_enc = np.asarray(inputs["b_enc"], np.float32)
    W_rel = np.asarray(inputs["W_rel"], np.float32)
    b_rel = np.asarray(inputs["b_rel"], np.float32)
    gaussian = np.asarray(inputs["gaussian_vectors"], np.float32)

    nc = _get_nc(use_ag)
    in_maps = _make_in_maps(x, W_enc, b_enc, W_rel, b_rel, gaussian, use_ag)
    res = run_bass_kernel_spmd(nc, in_maps, list(range(NC)), trace=trace)

    out = np.empty((N_WAY, D), np.float32)
    for c in range(NC):
        out[:, c * SL : (c + 1) * SL] = res.results[c]["out"].T
    return out, res


def kernel(**inputs) -> np.ndarray:
    out, _ = run(inputs)
    return out

